# revision 7
# baseline (speedup 1.0000x reference)
"""DiscriminativeLoss Trainium2 kernel (self-contained).

kernel(data, labels) -> np.float32 scalar loss.

Sharding: data-parallel over batch B=16 across 8 NeuronCores (2 items per
core). Per item the kernel computes segment sums/counts via packed one-hot
matmuls (PSUM-accumulated), cluster centers, the per-point hinge^2 total
via a one-hot gather matmul with the x-minus-centers subtraction folded
into the same PSUM accumulation (identity-matrix matmul), and the
center-pair distance/regularizer terms. The host converts inputs to
bf16/fp8/u8 and pre-arranges a stage-3 tile layout; the device averages
per-item losses into `oloss`, host means across cores.
"""

import numpy as np
from contextlib import ExitStack

import concourse.bass as bass
import concourse.tile as tile
import concourse.mybir as mybir

dt = mybir.dt
Alu = mybir.AluOpType
Act = mybir.ActivationFunctionType

C = 32
D = 8
DELTA_VAR = 1.0
DELTA_DIST = 2.0

B, H, W = 16, 512, 512
N_CORES = 8
NB = B // N_CORES
F = (H * W) // 128
N = 128 * F
OH_CHUNK = 512


def build_kernel(nc, F=2048, NB=2, oh_chunk=512, reps=1):
    N = 128 * F
    MPC = 32 * F                     # points per quadrant
    NSB = MPC // 2048                # superblocks per item
    n_groups = F
    n_acc = (NSB + 7) // 8
    s3_dt = dt.float8e4

    data_t = nc.dram_tensor("databf", [NB, D, N], dt.bfloat16,
                            kind="ExternalInput")
    datas3_t = nc.dram_tensor("datas3", [NB, NSB, 128, 512],
                              s3_dt, kind="ExternalInput")
    labs3_t = nc.dram_tensor("labs3", [NB, 4, MPC], dt.uint8,
                             kind="ExternalInput")
    onespad_d = nc.dram_tensor("onespad_c", [128, 240], dt.bfloat16,
                               kind="ExternalInput")
    iota32_d = nc.dram_tensor("iota32_c", [128, 1], dt.float32,
                              kind="ExternalInput")
    ident_d = nc.dram_tensor("ident_c", [128, 128], dt.bfloat16,
                             kind="ExternalInput")
    lab_t = nc.dram_tensor("labels", [NB, N], dt.uint8, kind="ExternalInput")
    osums_t = nc.dram_tensor("osums", [NB, C, 9], dt.float32,
                             kind="ExternalOutput")
    ohinge_t = nc.dram_tensor("ohinge", [1, NB], dt.float32,
                              kind="ExternalOutput")
    oloss_t = nc.dram_tensor("oloss", [1, NB], dt.float32,
                             kind="ExternalOutput")
    ibig_d = nc.dram_tensor("ibig_c", [C, C], dt.bfloat16,
                            kind="ExternalInput")
    data, lab, osums = data_t.ap(), lab_t.ap(), osums_t.ap()
    ohinge, oloss = ohinge_t.ap(), oloss_t.ap()
    datas3 = datas3_t.ap()

    with tile.TileContext(nc) as tc, ExitStack() as ctx:
        const_p = ctx.enter_context(tc.tile_pool(name="const", bufs=1))
        xbuf_p = ctx.enter_context(tc.tile_pool(name="xbuf", bufs=1))
        ld_p = ctx.enter_context(tc.tile_pool(name="ld", bufs=2))
        oh1_p = ctx.enter_context(tc.tile_pool(name="oh1", bufs=2))
        s3_p = ctx.enter_context(tc.tile_pool(name="s3", bufs=4))
        small_p = ctx.enter_context(tc.tile_pool(name="small", bufs=1))
        dram_p = ctx.enter_context(
            tc.tile_pool(name="dram", bufs=1, space=bass.MemorySpace.DRAM))
        ps_p = ctx.enter_context(
            tc.tile_pool(name="ps", bufs=1, space=bass.MemorySpace.PSUM))
        psg_p = ctx.enter_context(
            tc.tile_pool(name="psg", bufs=2, space=bass.MemorySpace.PSUM))
        pssm_p = ctx.enter_context(
            tc.tile_pool(name="pssm", bufs=1, space=bass.MemorySpace.PSUM))
        pssq_p = ctx.enter_context(
            tc.tile_pool(name="pssq", bufs=2, space=bass.MemorySpace.PSUM))

        # ---- constants ----
        iota32_bf = const_p.tile([128, 1], dt.float32)
        nc.sync.dma_start(iota32_bf[:], iota32_d.ap())
        onespad = const_p.tile([128, 240], dt.bfloat16)
        nc.sync.dma_start(onespad[:], onespad_d.ap())
        ident = const_p.tile([128, 128], dt.bfloat16)
        nc.sync.dma_start(ident[:], ident_d.ap())
        ident8 = const_p.tile([128, 128], dt.float8e4)
        nc.vector.tensor_copy(ident8[:], ident[:])
        ones_col = const_p.tile([128, 1], dt.bfloat16)
        nc.vector.memset(ones_col[:], 1.0)
        ibig = const_p.tile([C, C], dt.bfloat16)
        nc.sync.dma_start(ibig[:], ibig_d.ap())
        ones_row = const_p.tile([1, C], dt.bfloat16)
        nc.vector.memset(ones_row[:], 1.0)
        bias4 = const_p.tile([C, 1], dt.float32)
        nc.vector.memset(bias4[:], 2.0 * DELTA_DIST)
        negs8 = const_p.tile([C, 1], dt.float32)
        nc.vector.memset(negs8[:], -float(np.sqrt(D)))
        negdv = const_p.tile([128, 1], dt.float32)
        nc.vector.memset(negdv[:], -DELTA_VAR)

        for _rep in range(reps):
            hs_cols = small_p.tile([128, n_acc * NB], dt.float32, tag="hs")
            nc.vector.memset(hs_cols[:], 0.0)

            xalls, labbfs, ps_sums = [], [], []
            for b in range(NB):
                xalls.append(xbuf_p.tile([128, 9, F], dt.bfloat16,
                                         tag=f"xall{b}", name=f"xall{b}"))
                labbfs.append(xbuf_p.tile([128, F], dt.bfloat16,
                                          tag=f"labbf{b}", name=f"labbf{b}"))
                ps_sums.append(ps_p.tile([128, 36], dt.float32,
                                         tag=f"ps1_{b}", name=f"ps1_{b}"))

            # ============ loads + stage 1 ============
            def do_loads(b):
                xall, labbf = xalls[b], labbfs[b]
                for d in range(D):
                    nc.sync.dma_start(
                        xall[:, d, :],
                        data[b, d].rearrange("(p f) -> p f", p=128))
                nc.vector.memset(xall[:, 8, :], 1.0)
                li = ld_p.tile([128, F], dt.uint8, tag="li")
                nc.sync.dma_start(li[:],
                                  lab[b].rearrange("(p f) -> p f", p=128))
                nc.vector.tensor_copy(labbf[:], li[:])

            n_ch = n_groups // oh_chunk

            def do_stage1_chunk(b, ch):
                xall, labbf = xalls[b], labbfs[b]
                ps1 = ps_sums[b]
                oh1 = oh1_p.tile([128, oh_chunk // 4, 4 * C],
                                 dt.bfloat16, tag="oh1")
                for c in range(C):
                    nc.vector.tensor_scalar(
                        out=oh1[:, :, 4 * c:4 * c + 4],
                        in0=labbf[:, ch * oh_chunk:(ch + 1) * oh_chunk]
                        .rearrange("p (j t) -> p j t", t=4),
                        scalar1=float(c), scalar2=None, op0=Alu.is_equal)
                for jj in range(oh_chunk // 4):
                    J = ch * (oh_chunk // 4) + jj
                    nc.tensor.matmul(
                        ps1[:],
                        oh1[:, jj, :],
                        xall[:, :, 4 * J:4 * J + 4],
                        start=(J == 0),
                        stop=(J == n_groups // 4 - 1))

            # ============ stage 2: centers (negated for PE diff-fold) ====
            W4s, item_sc = {}, {}

            def do_stage2(b):
                ps1 = ps_sums[b]
                ps1sb = small_p.tile([128, 36], dt.float32, tag=f"ps1sb_{b}")
                nc.vector.tensor_copy(ps1sb[:], ps1[:])
                dfold = dram_p.tile([128, 36], dt.float32, tag=f"dfold_{b}",
                                    name=f"dfold_{b}")
                nc.sync.dma_start(dfold[:], ps1sb[:])
                s4 = small_p.tile([C, 4, 9], dt.float32, tag=f"s4_{b}")
                for t in range(4):
                    nc.sync.dma_start(
                        s4[:, t, :],
                        dfold[t:t + 125:4, t:t + 33:4])
                s01 = small_p.tile([C, 9], dt.float32, tag=f"s01_{b}")
                nc.vector.tensor_add(s01[:], s4[:, 0, :], s4[:, 1, :])
                s23 = small_p.tile([C, 9], dt.float32, tag=f"s23_{b}")
                nc.vector.tensor_add(s23[:], s4[:, 2, :], s4[:, 3, :])
                sums32 = small_p.tile([C, 9], dt.float32, tag=f"sums32_{b}")
                nc.vector.tensor_add(sums32[:], s01[:], s23[:])
                nc.sync.dma_start(osums[b], sums32[:])

                cnt = small_p.tile([C, 1], dt.float32, tag=f"cnt_{b}")
                nc.vector.tensor_scalar(
                    out=cnt[:], in0=sums32[:, 8:9], scalar1=1.0,
                    scalar2=None, op0=Alu.max)
                inv = small_p.tile([C, 1], dt.float32, tag=f"inv_{b}")
                nc.vector.reciprocal(inv[:], cnt[:])
                w2f = small_p.tile([C, D], dt.float32, tag=f"w2f_{b}")
                nc.vector.tensor_scalar(
                    out=w2f[:], in0=sums32[:, 0:8], scalar1=inv[:, 0:1],
                    scalar2=None, op0=Alu.mult)
                w2b = small_p.tile([C, D], dt.bfloat16, tag=f"w2b_{b}")
                nc.vector.tensor_copy(w2b[:], w2f[:])
                w2bn = small_p.tile([C, D], dt.bfloat16, tag=f"w2bn_{b}")
                nc.vector.tensor_scalar(
                    out=w2bn[:], in0=w2f[:], scalar1=-1.0, scalar2=None,
                    op0=Alu.mult)

                # W4[32q+c, 4d+q] = -mu_c[d]  (xL4 block rows are 4d+q)
                W4 = small_p.tile([128, C], dt.bfloat16, tag=f"W4_{b}")
                nc.vector.memset(W4[:], 0.0)
                for q in range(4):
                    nc.sync.dma_start(
                        W4[32 * q:32 * q + 32, q:q + 29:4], w2bn[:])
                W4s[b] = W4

                # ---- stage 4 per-item: dist/reg terms (tiny) ----
                sq8w = small_p.tile([C, D], dt.float32, tag=f"sq8w_{b}")
                nc.scalar.square(sq8w[:], w2f[:])
                msq = small_p.tile([C, 1], dt.float32, tag=f"msq_{b}")
                nc.vector.tensor_reduce(
                    out=msq[:], in_=sq8w[:], op=Alu.add,
                    axis=mybir.AxisListType.X)
                pres = small_p.tile([C, 1], dt.float32, tag=f"pres_{b}")
                nc.vector.tensor_scalar(
                    out=pres[:], in0=sums32[:, 8:9], scalar1=0.0,
                    scalar2=None, op0=Alu.is_gt)
                pres_bf = small_p.tile([C, 1], dt.bfloat16, tag=f"presb_{b}")
                nc.vector.tensor_copy(pres_bf[:], pres[:])
                msq_bf = small_p.tile([C, 1], dt.bfloat16, tag=f"msqb_{b}")
                nc.vector.tensor_copy(msq_bf[:], msq[:])
                dwb = dram_p.tile([C, D], dt.bfloat16, tag=f"dwb_{b}",
                                  name=f"dwb_{b}")
                nc.sync.dma_start(dwb[:], w2b[:])
                W2T = small_p.tile([D, C], dt.bfloat16, tag=f"W2T_{b}")
                nc.sync.dma_start(W2T[:], dwb[:].rearrange("c d -> d c"))
                dmq = dram_p.tile([C, 1], dt.bfloat16, tag=f"dmq_{b}",
                                  name=f"dmq_{b}")
                nc.sync.dma_start(dmq[:], msq_bf[:])
                msq_row = small_p.tile([1, C], dt.bfloat16, tag=f"msqr_{b}")
                nc.sync.dma_start(msq_row[:], dmq[:].rearrange("c one -> one c"))
                W2Tm2 = small_p.tile([D, C], dt.bfloat16, tag=f"W2Tm2_{b}")
                nc.vector.tensor_scalar(
                    out=W2Tm2[:], in0=W2T[:], scalar1=-2.0, scalar2=None,
                    op0=Alu.mult)
                pscsq = pssm_p.tile([C, C], dt.float32, tag="pssm")
                nc.tensor.matmul(pscsq[:], W2Tm2[:], W2T[:],
                                 start=True, stop=False)
                nc.tensor.matmul(pscsq[:], msq_row[:], ones_row[:],
                                 start=False, stop=False)
                nc.tensor.matmul(pscsq[:], ones_row[:], msq_row[:],
                                 start=False, stop=False)
                nc.tensor.matmul(pscsq[:], ibig[:], ibig[:],
                                 start=False, stop=True)
                cdist = small_p.tile([C, C], dt.float32, tag=f"cdist_{b}")
                nc.scalar.sqrt(cdist[:], pscsq[:])
                hz = small_p.tile([C, C], dt.float32, tag=f"hz_{b}")
                nc.scalar.activation(hz[:], cdist[:], Act.Relu,
                                     bias=bias4[:, 0:1], scale=-1.0)
                dh_bf = small_p.tile([C, C], dt.bfloat16, tag=f"dhb_{b}")
                nc.scalar.square(dh_bf[:], hz[:])
                psv = pssm_p.tile([C, 1], dt.float32, tag="psvT", name="psv")
                nc.tensor.matmul(psv[:], dh_bf[:], pres_bf[:],
                                 start=True, stop=True)
                v_bf = small_p.tile([C, 1], dt.bfloat16, tag=f"vb_{b}")
                nc.vector.tensor_copy(v_bf[:], psv[:])
                psT = pssm_p.tile([1, 4], dt.float32, tag="psvT", name="psT")
                nc.tensor.matmul(psT[:, 0:1], v_bf[:], pres_bf[:],
                                 start=True, stop=True)
                cn = small_p.tile([C, 1], dt.float32, tag=f"cn_{b}")
                nc.scalar.sqrt(cn[:], msq[:])
                rg = small_p.tile([C, 1], dt.float32, tag=f"rg_{b}")
                nc.scalar.activation(rg[:], cn[:], Act.Relu,
                                     bias=negs8[:, 0:1], scale=1.0)
                rg_bf = small_p.tile([C, 1], dt.bfloat16, tag=f"rgb_{b}")
                nc.vector.tensor_copy(rg_bf[:], rg[:])
                nc.tensor.matmul(psT[:, 1:2], rg_bf[:], pres_bf[:],
                                 start=True, stop=True)
                nc.tensor.matmul(psT[:, 2:3], pres_bf[:], ones_col[0:C, :],
                                 start=True, stop=True)
                tks = small_p.tile([1, 3], dt.float32, tag=f"tks_{b}",
                                   name=f"tks_{b}")
                nc.vector.tensor_copy(tks[:], psT[:, 0:3])
                item_sc[b] = tks

            # ============ stage 3 ============
            sqbanks = {}

            def do_stage3_sb(b, s):
                W4 = W4s[b]
                sqbank = sqbanks.get(b)
                labrep = s3_p.tile([128, 2048], dt.uint8, tag="labrep")
                lsrc = labs3_t.ap()[b][:, s * 2048:(s + 1) * 2048]
                lsrc = lsrc.unsqueeze(1).broadcast_to([4, 32, 2048])
                # alternate broadcast between Pool (SWDGE) and SP queues
                (nc.gpsimd.dma_start if s % 2 == 0
                 else nc.sync.dma_start)(labrep[:], lsrc)
                oht = s3_p.tile([128, 2048], dt.bfloat16, tag="oht")
                nc.vector.tensor_scalar(
                    out=oht[:], in0=labrep[:], scalar1=iota32_bf[:, 0:1],
                    scalar2=None, op0=Alu.is_equal)

                xL4 = s3_p.tile([128, 512], dt.float8e4, tag="xL4")
                nc.sync.dma_start(xL4[:], datas3[b, s])

                g4 = psg_p.tile([128, 512], dt.float32, tag="g4")
                for j in range(4):
                    nc.tensor.matmul(
                        g4[32 * j:32 * (j + 1), :],
                        W4[:],
                        oht[:, j * 512:(j + 1) * 512],
                        start=True, stop=False,
                        tile_position=(0, 32 * j),
                        skip_group_check=True)
                # accumulate +x into the same PSUM: g4 = x - centers
                nc.tensor.matmul(g4[:], ident8[:], xL4[:],
                                 start=False, stop=True,
                                 skip_group_check=True)

                sq8 = s3_p.tile([128, 512], dt.bfloat16, tag="sq8")
                nc.scalar.square(sq8[:], g4[:])

                v = s % 8
                if v == 0:
                    sqbank = pssq_p.tile([128, 512], dt.float32,
                                         tag="sqbank")
                    sqbanks[b] = sqbank
                nc.tensor.matmul(
                    sqbank[:], onespad[:, 112 - 16 * v:240 - 16 * v],
                    sq8[:],
                    start=(v == 0), stop=(v == 7 or s == NSB - 1))

                if v == 7 or s == NSB - 1:
                    nrow = 16 * (v + 1)
                    acc_i = s // 8
                    col = b * n_acc + acc_i
                    dist = s3_p.tile([128, 512], dt.bfloat16, tag="dist")
                    nc.scalar.sqrt(dist[:nrow, :], sqbank[:nrow, :])
                    hin = s3_p.tile([128, 512], dt.bfloat16, tag="hin")
                    nc.scalar.activation(
                        hin[:nrow, :], dist[:nrow, :], Act.Relu,
                        bias=negdv[:nrow, 0:1], scale=1.0)
                    hsq = s3_p.tile([128, 512], dt.bfloat16, tag="hsq")
                    nc.scalar.activation(
                        hsq[:nrow, :], hin[:nrow, :], Act.Square,
                        accum_out=hs_cols[:nrow, col:col + 1])

            # ---- issue order: interleave s1(b+1) with s3(b) ----
            if NB == 2:
                do_loads(0)
                do_loads(1)
                for ch in range(n_ch):
                    do_stage1_chunk(0, ch)
                do_stage2(0)
                sb_per_ch = NSB // n_ch
                for ch in range(n_ch):
                    do_stage1_chunk(1, ch)
                    for s in range(ch * sb_per_ch, (ch + 1) * sb_per_ch):
                        do_stage3_sb(0, s)
                do_stage2(1)
                for s in range(NSB):
                    do_stage3_sb(1, s)
            else:
                for b in range(NB):
                    do_loads(b)
                    for ch in range(n_ch):
                        do_stage1_chunk(b, ch)
                    do_stage2(b)
                for b in range(NB):
                    for s in range(NSB):
                        do_stage3_sb(b, s)

            # ============ hinge partition reduce ============
            hsb = small_p.tile([128, n_acc * NB], dt.bfloat16, tag="hsb")
            nc.vector.tensor_copy(hsb[:], hs_cols[:])
            pssm = pssm_p.tile([1, n_acc * NB], dt.float32, tag="pssm")
            nc.tensor.matmul(pssm[:], ones_col[:], hsb[:], start=True,
                             stop=True)
            psm_sb = small_p.tile([1, n_acc * NB], dt.float32, tag="psm_sb")
            nc.vector.tensor_copy(psm_sb[:], pssm[:])
            hview = small_p.tile([1, NB], dt.float32, tag="hview")
            if n_acc == 1:
                nc.vector.tensor_copy(hview[:], psm_sb[:])
            else:
                acc = small_p.tile([1, NB], dt.float32, tag="hacc")
                nc.vector.tensor_add(
                    acc[:],
                    psm_sb[:].rearrange("p (b a) -> p b a", a=n_acc)[:, :, 0],
                    psm_sb[:].rearrange("p (b a) -> p b a", a=n_acc)[:, :, 1])
                for a in range(2, n_acc):
                    nxt = small_p.tile([1, NB], dt.float32, tag=f"hacc{a}")
                    nc.vector.tensor_add(
                        nxt[:], acc[:],
                        psm_sb[:].rearrange("p (b a) -> p b a",
                                            a=n_acc)[:, :, a])
                    acc = nxt
                nc.vector.tensor_copy(hview[:], acc[:])
            nc.sync.dma_start(ohinge[:], hview[:])

            # ---- stage 4 combine: per-item loss ----
            lossr = small_p.tile([1, NB], dt.float32, tag="lossr")
            for b in range(NB):
                tks = item_sc[b]   # [1, 3]: cols 0=dist_sum, 1=reg, 2=K
                kk = small_p.tile([1, 4], dt.float32, tag=f"kk_{b}",
                                  name=f"kk_{b}")
                nc.vector.tensor_copy(kk[:, 0:1], tks[:, 2:3])
                nc.vector.tensor_scalar(
                    out=kk[:, 1:2], in0=tks[:, 2:3], scalar1=-1.0,
                    scalar2=1.0, op0=Alu.add, op1=Alu.max)
                rki = small_p.tile([1, 2], dt.float32, tag=f"rki_{b}",
                                   name=f"rki_{b}")
                nc.vector.reciprocal(rki[:], kk[:, 0:2])
                t0 = small_p.tile([1, 4], dt.float32, tag=f"t0_{b}",
                                  name=f"t0_{b}")
                nc.vector.tensor_mul(t0[:, 0:1], hview[:, b:b + 1],
                                     rki[:, 0:1])
                nc.vector.tensor_mul(t0[:, 1:2], tks[:, 0:1], rki[:, 0:1])
                nc.vector.tensor_mul(t0[:, 2:3], t0[:, 1:2], rki[:, 1:2])
                nc.vector.tensor_mul(t0[:, 3:4], tks[:, 1:2], rki[:, 0:1])
                t1 = small_p.tile([1, 1], dt.float32, tag=f"t1_{b}",
                                  name=f"t1_{b}")
                nc.vector.tensor_scalar(
                    out=t1[:], in0=t0[:, 2:3], scalar1=0.5, scalar2=None,
                    op0=Alu.mult)
                t2 = small_p.tile([1, 1], dt.float32, tag=f"t2_{b}",
                                  name=f"t2_{b}")
                nc.vector.tensor_add(t2[:], t0[:, 0:1], t1[:])
                t3 = small_p.tile([1, 1], dt.float32, tag=f"t3_{b}",
                                  name=f"t3_{b}")
                nc.vector.tensor_add(t3[:], t2[:], t0[:, 3:4])
                gk = small_p.tile([1, 1], dt.float32, tag=f"gk_{b}",
                                  name=f"gk_{b}")
                nc.vector.tensor_scalar(
                    out=gk[:], in0=kk[:, 0:1], scalar1=1.0, scalar2=None,
                    op0=Alu.is_gt)
                nc.vector.tensor_mul(lossr[:, b:b + 1], t3[:], gk[:])
            nc.sync.dma_start(oloss[:], lossr[:])

    return nc


def make_consts():
    import ml_dtypes
    onespad = np.zeros((128, 240), ml_dtypes.bfloat16)
    for j in range(4):
        for q in range(4):
            for d in range(8):
                onespad[32 * j + 4 * d + q, 112 + 4 * j + q] = 1.0
    iota32 = (np.arange(128, dtype=np.float32) % 32).reshape(128, 1)
    ibig = (np.eye(C, dtype=np.float32) * 100.0).astype(ml_dtypes.bfloat16)
    ident = np.eye(128, dtype=np.float32).astype(ml_dtypes.bfloat16)
    return {"onespad_c": onespad, "iota32_c": iota32, "ibig_c": ibig,
            "ident_c": ident}


def make_in_maps(data, labels, n_cores=N_CORES, nb=NB):
    """data [16,8,512,512] f32, labels [16,512,512] int -> per-core maps."""
    import ml_dtypes
    B_ = n_cores * nb
    N_ = data.shape[2] * data.shape[3]
    NSB = N_ // 4 // 2048
    MPC = N_ // 4
    databf = np.asarray(data, np.float32).astype(
        ml_dtypes.bfloat16).reshape(B_, D, N_)
    datas3 = np.ascontiguousarray(
        np.asarray(data, np.float32).reshape(B_, D, 4, NSB, 4, 512)
        .transpose(0, 3, 4, 1, 2, 5)
        .reshape(B_, NSB, 128, 512).astype(ml_dtypes.float8_e4m3))
    labu8 = np.asarray(labels).astype(np.uint8).reshape(B_, N_)
    consts = make_consts()
    in_maps = []
    for i in range(n_cores):
        sl = slice(nb * i, nb * (i + 1))
        in_maps.append({
            "databf": databf[sl],
            "datas3": datas3[sl],
            "labels": labu8[sl],
            "labs3": labu8[sl].reshape(nb, 4, MPC),
            **consts,
        })
    return in_maps


_COMPILED = {}


def _get_compiled():
    if "nc" not in _COMPILED:
        from concourse import bacc
        nc = bacc.Bacc("TRN2", target_bir_lowering=False, debug=False,
                       num_devices=N_CORES)
        build_kernel(nc, F=F, NB=NB, oh_chunk=OH_CHUNK)
        nc.compile()
        _COMPILED["nc"] = nc
    return _COMPILED["nc"]


def kernel(data, labels):
    """data [16,8,512,512] f32, labels [16,512,512] int -> scalar f32 loss."""
    from concourse.bass_utils import run_bass_kernel_spmd

    data = np.ascontiguousarray(np.asarray(data, dtype=np.float32))
    labels = np.ascontiguousarray(np.asarray(labels))
    assert data.shape == (B, D, H, W), data.shape
    assert labels.shape == (B, H, W), labels.shape

    nc = _get_compiled()
    in_maps = make_in_maps(data, labels)
    # retry guards: the very first execution after a cold compile has been
    # observed to produce garbage once; a warm re-run is reliable. If the
    # device reports unrecoverable, wait for the runtime to reset it.
    import time as _time
    per_item = None
    for _attempt in range(3):
        try:
            res = run_bass_kernel_spmd(nc, in_maps, list(range(N_CORES)))
        except Exception:
            if _attempt == 2:
                raise
            _time.sleep(75)
            continue
        per_item = np.concatenate(
            [res.results[i]["oloss"][0] for i in range(N_CORES)])
        if np.all(np.isfinite(per_item)):
            break
    return np.array(np.mean(per_item), dtype=np.float32)
